# revision 37
# baseline (speedup 1.0000x reference)
"""Trainium2 Bass kernel for an Attention + dense-MoE transformer layer.

Distribution: pure data-parallel over the batch dim (B=8) across 8
NeuronCores — one batch element per core, weights replicated. The dense
MoE (every token through every expert, gate-weighted sum) means compute
is identical under any sharding; DP avoids all collectives.

Per-core pipeline (S=1024 tokens, D=1024, H=16 heads, F=4096, E=8):
  rmsnorm1 -> (PE-transpose) xnT -> QKV (q,k feature-major; v token-major)
  -> per-head scoresT = k_h^T.T@q_h^T -> exp (no max; values bounded)
  -> denom via ones-matmul (cross-partition sum, broadcast) -> av matmul
  -> scale by 1/denom -> Wo proj + residual -> rmsnorm2 -> x1nT
  -> gate softmax -> per-expert h=relu(x1n@W1e+b1), y=h@W2e, acc += g*(y+b2)
  -> out = acc (accumulated in-place on x1).

Attention matmuls run in bf16 (precision-sensitive: the attention output
feeds the residual stream directly). The MoE matmuls — 86% of all PE
cycles — run in fp8 e4m3 with MatmulPerfMode.DoubleRow (two k-tiles per
pass, 2x PE throughput). W1 is pre-scaled x64 and W2 x128 on cast so the
fp8 values sit in e4m3's normal range; the scales are folded back via
the relu activation scale (1/64) and the gate coefficient (gate/128).
Numpy-simulated end-to-end rel err: 1.70e-2 (gate 2e-2); the same
simulator reproduces the bf16 baseline's HW 1.257e-3 to 3 decimals.

Weights are DMA'd f32 with row-contiguous 2-4KB lines (plain
W[kt*128:(kt+1)*128, :] block slices land partition=row with no strided
rearrange), then cast on-chip: QKVO on DVE/ACT, W1/W2 fp8 casts on the
otherwise-idle GpSimd so they never head-of-line-block the relu/exp
streams. W1 is double-buffered across experts and W2 streamed as d-halves
(3 rotating buffers) so each expert's weights land before its matmuls.
Residual stream, norms, psum accum stay f32.
"""
import sys

if '/opt/trn_rl_repo' not in sys.path:
    sys.path.insert(0, '/opt/trn_rl_repo')

import numpy as np

import concourse.bass as bass
import concourse.tile as tile
from concourse import bacc, mybir
from concourse.masks import make_identity
from concourse.bass_utils import run_bass_kernel_spmd

F32 = mybir.dt.float32
BF16 = mybir.dt.bfloat16
FP8 = mybir.dt.float8e4
AX = mybir.AxisListType.X
AF = mybir.ActivationFunctionType
DR = mybir.MatmulPerfMode.DoubleRow

B, S, D, H, F, E = 8, 1024, 1024, 16, 4096, 8
DH = D // H            # 64 head dim
ST = S // 128          # 8 token tiles
DT = D // 128          # 8 feature tiles
FT = F // 128          # 32 ffn tiles
EPS = 1e-6
SCALE = DH ** -0.5     # 0.125
NCORES = 8
CH = 512               # attention s_q chunk
NCH = S // CH          # 2
W1SC = 64.0            # fp8 pre-scale for W1 (elem std 1/32 -> 2)
W2SC = 128.0           # fp8 pre-scale for W2 (elem std 1/64 -> 2)
NST = 2                # experts pre-converted to fp8 in DRAM scratch
                       # during the attention phase (its DMA is ~95% idle);
                       # each staged expert cuts MoE-phase DMA 32MB -> 8MB


def build():
    nc = bacc.Bacc("TRN2", target_bir_lowering=False)

    x = nc.declare_dram_parameter("x", [S, D], F32, isOutput=False)
    g1 = nc.declare_dram_parameter("g1", [D], F32, isOutput=False)
    Wq = nc.declare_dram_parameter("Wq", [D, D], F32, isOutput=False)
    Wk = nc.declare_dram_parameter("Wk", [D, D], F32, isOutput=False)
    Wv = nc.declare_dram_parameter("Wv", [D, D], F32, isOutput=False)
    Wo = nc.declare_dram_parameter("Wo", [D, D], F32, isOutput=False)
    g2 = nc.declare_dram_parameter("g2", [D], F32, isOutput=False)
    Wg = nc.declare_dram_parameter("Wg", [D, E], F32, isOutput=False)
    W1 = nc.declare_dram_parameter("W1", [E, D, F], F32, isOutput=False)
    b1 = nc.declare_dram_parameter("b1", [E, F], F32, isOutput=False)
    W2 = nc.declare_dram_parameter("W2", [E, F, D], F32, isOutput=False)
    b2 = nc.declare_dram_parameter("b2", [E, D], F32, isOutput=False)
    out = nc.declare_dram_parameter("out", [S, D], F32, isOutput=True)

    with tile.TileContext(nc) as tc:
        with tc.tile_pool(name="pers", bufs=1) as pers, \
             tc.tile_pool(name="x8p", bufs=1) as x8p, \
             tc.tile_pool(name="dramp", bufs=1, space="DRAM") as dramp:

            # DRAM fp8 scratch for the pre-staged experts, laid out exactly
            # as the SBUF tiles the MoE phase fills, so both the conversion
            # writes and the MoE reads use large contiguous lines.
            w1d = [dramp.tile([128, DT * F], FP8, name=f"w1d{e}")
                   for e in range(NST)]
            w2d = [[dramp.tile([128, FT * 512], FP8, name=f"w2d{e}n{nh}")
                    for nh in range(2)] for e in range(NST)]

            # ---- persistent setup ----
            x_sb = pers.tile([128, ST, D], F32)
            xr = x.ap().rearrange("(st p) d -> p st d", p=128)
            for st in range(ST):
                nc.sync.dma_start(out=x_sb[:, st, :], in_=xr[:, st, :])
            ident = pers.tile([128, 128], F32)
            make_identity(nc, ident)
            ones_bf = pers.tile([128, 128], BF16)
            nc.vector.memset(ones_bf, 1.0)
            eps_sb = pers.tile([128, 1], F32)
            nc.vector.memset(eps_sb, EPS)
            wg_sb = pers.tile([128, DT, E], BF16)
            nc.gpsimd.dma_start(out=wg_sb, in_=Wg.ap().rearrange("(kt p) e -> p kt e", p=128))
            b1T = pers.tile([128, FT, E], F32)
            gate_sb = pers.tile([128, ST, E], F32)
            gate8 = pers.tile([128, ST, E], F32)   # gate / W2SC
            x8 = x8p.tile([128, DT, S], FP8)

            # ============== Scope I: attention + gate ==============
            # Pool creation order matters: qT/kT/v die earliest (end of
            # attention core), so they sit base-most — scope II's W1/W2
            # pools land on top of them and their prefetch DMA can start
            # while the tail of scope I still runs.
            with tc.tile_pool(name="attnA", bufs=1) as attnA, \
                 tc.tile_pool(name="attnB", bufs=1) as attnB, \
                 tc.tile_pool(name="x1p", bufs=1) as x1p, \
                 tc.tile_pool(name="small", bufs=4) as small, \
                 tc.tile_pool(name="gpool", bufs=1) as gpool:
                qT = attnA.tile([128, DT, S], BF16, tag="qT")
                kT = attnA.tile([128, DT, S], BF16, tag="kT")
                v_sb = attnA.tile([128, ST, D], BF16, tag="v")
                xnT = attnB.tile([128, DT, S], BF16, tag="xT")
                x1nT = x1p.tile([128, DT, S], BF16)
                gc1 = gpool.tile([128, DT], F32)
                nc.sync.dma_start(out=gc1, in_=g1.ap().rearrange("(dt p) -> p dt", p=128))
                gc2 = gpool.tile([128, DT], F32)
                nc.sync.dma_start(out=gc2, in_=g2.ap().rearrange("(dt p) -> p dt", p=128))

                def rmsnorm_transpose(gcol, dstT, ps_tp, tmp):
                    """token-major rmsnorm of x_sb, PE-transposed into dstT
                    [128, DT, S] (bf16, feature-major); the gain g is folded
                    into the transpose drain, alternating DVE/ACT."""
                    for st in range(ST):
                        xs = x_sb[:, st, :]
                        sq = tmp.tile([128, D], F32, tag="scr")
                        ss = small.tile([128, 1], F32, tag="ss")
                        # one ACT op: squares + free-axis sum via accum_out
                        nc.scalar.activation(sq, xs, AF.Square, accum_out=ss)
                        rstd = small.tile([128, 1], F32, tag="rstd")
                        nc.scalar.activation(rstd, ss, AF.Sqrt, bias=eps_sb, scale=1.0 / D)
                        rs = small.tile([128, 1], F32, tag="rs")
                        nc.vector.reciprocal(rs, rstd)
                        xn = tmp.tile([128, D], F32, tag="scr")
                        nc.vector.tensor_scalar_mul(xn, xs, rs)
                        for dt_ in range(DT):
                            tp = ps_tp.tile([128, 128], F32, tag="tp")
                            nc.tensor.transpose(tp, xn[:, dt_ * 128:(dt_ + 1) * 128], ident)
                            dslice = dstT[:, dt_, st * 128:(st + 1) * 128]
                            if dt_ % 2 == 0:
                                nc.vector.tensor_scalar_mul(
                                    dslice, tp, gcol[:, dt_:dt_ + 1])
                            else:
                                nc.scalar.activation(
                                    dslice, tp, AF.Copy, scale=gcol[:, dt_:dt_ + 1])

                def load_wproj(wh, wpool, spool):
                    """Row-contiguous load of a [D, D] weight into bf16
                    [128, DT, D] (partition = row-in-block). 4KB DMA lines,
                    no strided rearrange."""
                    wbf = wpool.tile([128, DT, D], BF16, tag="w")
                    for kt in range(DT):
                        ws = spool.tile([128, D], F32, tag="ws")
                        nc.sync.dma_start(
                            out=ws, in_=wh[kt * 128:(kt + 1) * 128, :])
                        if kt % 2 == 0:
                            nc.vector.tensor_copy(wbf[:, kt, :], ws)
                        else:
                            nc.scalar.activation(wbf[:, kt, :], ws, AF.Copy)
                    return wbf

                # ---- rmsnorm1, b1 transpose, QKV ----
                with tc.tile_pool(name="ps12", bufs=3, space="PSUM") as ps12, \
                     tc.tile_pool(name="wpp", bufs=2) as wpp, \
                     tc.tile_pool(name="wstg", bufs=4) as wstg, \
                     tc.tile_pool(name="tmp1", bufs=3) as tmp1, \
                     tc.tile_pool(name="b1rp", bufs=1) as b1rp:
                    # b1 -> b1T via PE transpose (8 contiguous DMA lines
                    # instead of 32K 4-byte strided lines)
                    b1row = b1rp.tile([E, F], F32)
                    nc.gpsimd.dma_start(out=b1row, in_=b1.ap())
                    for ft in range(FT):
                        tpb = ps12.tile([128, 128], F32, tag="tp")
                        nc.tensor.transpose(
                            tpb[:, 0:E], b1row[:, ft * 128:(ft + 1) * 128],
                            ident[0:E, 0:E])
                        nc.vector.tensor_copy(b1T[:, ft, :], tpb[:, 0:E])

                    rmsnorm_transpose(gc1, xnT, ps12, tmp1)

                    for wh, dstT in ((Wq, qT), (Wk, kT)):
                        wbf = load_wproj(wh, wpp, wstg)
                        for mt in range(DT):
                            for nh in range(2):
                                ps = ps12.tile([128, 512], F32, tag="mm")
                                for kt in range(DT):
                                    nc.tensor.matmul(
                                        ps, wbf[:, kt, mt * 128:(mt + 1) * 128],
                                        xnT[:, kt, nh * 512:(nh + 1) * 512],
                                        start=(kt == 0), stop=(kt == DT - 1))
                                dsl = dstT[:, mt, nh * 512:(nh + 1) * 512]
                                if (2 * mt + nh) % 2 == 0:
                                    nc.vector.tensor_copy(dsl, ps)
                                else:
                                    nc.scalar.activation(dsl, ps, AF.Copy)

                    wv_c = load_wproj(Wv, wpp, wstg)
                    for nh in range(2):
                        for st in range(ST):
                            ps = ps12.tile([128, 512], F32, tag="mm")
                            for kt in range(DT):
                                nc.tensor.matmul(
                                    ps, xnT[:, kt, st * 128:(st + 1) * 128],
                                    wv_c[:, kt, nh * 512:(nh + 1) * 512],
                                    start=(kt == 0), stop=(kt == DT - 1))
                            vsl = v_sb[:, st, nh * 512:(nh + 1) * 512]
                            if st % 2 == 0:
                                nc.vector.tensor_copy(vsl, ps)
                            else:
                                nc.scalar.activation(vsl, ps, AF.Copy)

                # ---- attention core ----
                # Head PAIRS (2t, 2t+1) share one 128-row tile of qT/kT:
                # even head in partitions 0-63, odd in 64-127. dn/av stack
                # the pair on psum row groups 0/64. Software-pipelined:
                # iteration i's scores are emitted before iteration i-1's
                # dn/av so the PE never waits on ACT's exp.
                avT = attnB.tile([128, DT, S], BF16, tag="xT")  # reuses xnT
                with tc.tile_pool(name="wop", bufs=1) as wop, \
                     tc.tile_pool(name="wstg2", bufs=2) as wstg2:
                    with tc.tile_pool(name="ps3", bufs=2, space="PSUM") as ps3, \
                         tc.tile_pool(name="expp", bufs=2) as expp, \
                         tc.tile_pool(name="recp", bufs=2) as recp, \
                         tc.tile_pool(name="cstg", bufs=3) as cstg, \
                         tc.tile_pool(name="cq8", bufs=3) as cq8:
                        # Wo load emitted first: its DMA+casts overlap the
                        # attention core, so the Wo matmuls start immediately
                        # after the last av drain.
                        wo_c = load_wproj(Wo, wop, wstg2)

                        # fp8 pre-staging of experts 0..NST-1 into DRAM
                        # scratch: f32 in, DVE cast, fp8 out. Emitted a few
                        # units per attention iteration so the casts never
                        # delay the PE-critical recip/mul drain by much.
                        # Conversion DMA rides the GpSimd trigger queue:
                        # the sync queue carries the MoE weight stream, and
                        # an in-order queue would make the MoE prologue wait
                        # behind any conversion traffic that outlives the
                        # attention core.
                        def conv_w1_unit(e, c):
                            kt, q = c // 4, c % 4
                            fs = slice(q * 1024, (q + 1) * 1024)
                            cs = cstg.tile([128, 1024], F32, tag="cs", name="cs")
                            nc.gpsimd.dma_start(
                                out=cs, in_=W1[e, kt * 128:(kt + 1) * 128, fs])
                            c8 = cq8.tile([128, 1024], FP8, tag="c8", name="c8")
                            nc.vector.tensor_scalar_mul(c8, cs, W1SC)
                            nc.gpsimd.dma_start(
                                out=w1d[e][:, kt * F + q * 1024:
                                           kt * F + (q + 1) * 1024], in_=c8)

                        def conv_w2_unit(e, wc):
                            cs = cstg.tile([128, 1024], F32, tag="cs", name="cs")
                            nc.gpsimd.dma_start(
                                out=cs, in_=W2[e, wc * 128:(wc + 1) * 128, :])
                            c8 = cq8.tile([128, 1024], FP8, tag="c8", name="c8")
                            nc.vector.tensor_scalar_mul(c8, cs, W2SC)
                            for nh2 in range(2):
                                nc.gpsimd.dma_start(
                                    out=w2d[e][nh2][:, wc * 512:(wc + 1) * 512],
                                    in_=c8[:, nh2 * 512:(nh2 + 1) * 512])

                        conv_units = []
                        for e in range(NST):
                            conv_units += [(conv_w1_unit, e, c) for c in range(32)]
                            conv_units += [(conv_w2_unit, e, wc) for wc in range(FT)]
                        conv_pos = 0

                        def conv_step(n):
                            nonlocal conv_pos
                            for fn, e, i in conv_units[conv_pos:conv_pos + n]:
                                fn(e, i)
                            conv_pos += n

                        def attn_drain(state):
                            t, cs, exp_e, exp_o = state
                            ps_dn = ps3.tile([128, CH], F32, tag="dn")
                            for kt in range(ST):
                                nc.tensor.matmul(
                                    ps_dn[0:64, :], ones_bf[:, 0:64], exp_e[:, kt, :],
                                    start=(kt == 0), stop=(kt == ST - 1))
                                nc.tensor.matmul(
                                    ps_dn[64:128, :], ones_bf[:, 64:128], exp_o[:, kt, :],
                                    start=(kt == 0), stop=(kt == ST - 1))
                            recipb = recp.tile([128, CH], F32, tag="recip")
                            nc.vector.reciprocal_approx_fast(recipb, ps_dn)
                            ps_av = ps3.tile([128, CH], F32, tag="av")
                            for kt in range(ST):
                                nc.tensor.matmul(
                                    ps_av[0:64, :],
                                    v_sb[:, kt, (2 * t) * 64:(2 * t) * 64 + 64],
                                    exp_e[:, kt, :],
                                    start=(kt == 0), stop=(kt == ST - 1))
                                nc.tensor.matmul(
                                    ps_av[64:128, :],
                                    v_sb[:, kt, (2 * t + 1) * 64:(2 * t + 1) * 64 + 64],
                                    exp_o[:, kt, :],
                                    start=(kt == 0), stop=(kt == ST - 1))
                            nc.vector.tensor_mul(avT[:, t, cs], ps_av, recipb)

                        prev = None
                        for t in range(H // 2):
                            for c in range(NCH):
                                cs = slice(c * CH, (c + 1) * CH)
                                exp_e = expp.tile([128, ST, CH], BF16, tag="expe")
                                exp_o = expp.tile([128, ST, CH], BF16, tag="expo")
                                for kt in range(ST):
                                    ks = slice(kt * 128, (kt + 1) * 128)
                                    ps_e = ps3.tile([128, CH], F32, tag="sce")
                                    nc.tensor.matmul(
                                        ps_e, kT[0:64, t, ks], qT[0:64, t, cs],
                                        start=True, stop=True)
                                    ps_o = ps3.tile([128, CH], F32, tag="sco")
                                    nc.tensor.matmul(
                                        ps_o, kT[64:128, t, ks], qT[64:128, t, cs],
                                        start=True, stop=True)
                                    nc.scalar.activation(
                                        exp_e[:, kt, :], ps_e, AF.Exp, scale=SCALE)
                                    nc.scalar.activation(
                                        exp_o[:, kt, :], ps_o, AF.Exp, scale=SCALE)
                                cur = (t, cs, exp_e, exp_o)
                                if prev is not None:
                                    attn_drain(prev)
                                conv_step((len(conv_units) + 31) // 32)
                                prev = cur
                        attn_drain(prev)
                        conv_step(len(conv_units) - conv_pos)

                    # ---- Wo proj + residual, rmsnorm2, gate ----
                    with tc.tile_pool(name="ps4", bufs=3, space="PSUM") as ps4, \
                         tc.tile_pool(name="tmp2", bufs=3) as tmp2:
                        for nh in range(2):
                            for st in range(ST):
                                ps = ps4.tile([128, 512], F32, tag="mm")
                                for kt in range(DT):
                                    nc.tensor.matmul(
                                        ps, avT[:, kt, st * 128:(st + 1) * 128],
                                        wo_c[:, kt, nh * 512:(nh + 1) * 512],
                                        start=(kt == 0), stop=(kt == DT - 1))
                                nc.vector.tensor_add(
                                    x_sb[:, st, nh * 512:(nh + 1) * 512],
                                    x_sb[:, st, nh * 512:(nh + 1) * 512], ps)

                        rmsnorm_transpose(gc2, x1nT, ps4, tmp2)

                        # gate = softmax(x1n @ Wg) token-major [128, st, E]
                        for st in range(ST):
                            ps = ps4.tile([128, 512], F32, tag="mm")
                            for kt in range(DT):
                                nc.tensor.matmul(
                                    ps[:, :E], x1nT[:, kt, st * 128:(st + 1) * 128],
                                    wg_sb[:, kt, :],
                                    start=(kt == 0), stop=(kt == DT - 1))
                            gexp = small.tile([128, E], F32, tag="gexp")
                            nc.scalar.activation(gexp, ps[:, :E], AF.Exp)
                            gsum = small.tile([128, 1], F32, tag="gsum")
                            nc.vector.reduce_sum(gsum, gexp, axis=AX)
                            grec = small.tile([128, 1], F32, tag="grec")
                            nc.vector.reciprocal(grec, gsum)
                            nc.vector.tensor_scalar_mul(gate_sb[:, st, :], gexp, grec)
                            nc.vector.tensor_scalar_mul(
                                gate8[:, st, :], gate_sb[:, st, :], 1.0 / W2SC)

                        # x1nT's last reader is the gate matmul; cast to fp8
                        # now so the x1p region frees early for scope II.
                        nc.vector.tensor_copy(x8, x1nT)

                        # out += gate @ b2 (handles Sum_e g_e*b2_e once)
                        b2rb = gpool.tile([8, D], BF16)
                        nc.gpsimd.dma_start(out=b2rb, in_=b2.ap())
                        gateT = gpool.tile([8, ST, 128], BF16)
                        for st in range(ST):
                            tpg = ps4.tile([128, 128], F32, tag="tp")
                            nc.tensor.transpose(
                                tpg[:8, :], gate_sb[:, st, :], ident)
                            nc.vector.tensor_copy(gateT[:, st, :], tpg[:8, :])
                        for st in range(ST):
                            for nh in range(2):
                                ps = ps4.tile([128, 512], F32, tag="mm")
                                nc.tensor.matmul(
                                    ps, gateT[:, st, :],
                                    b2rb[:, nh * 512:(nh + 1) * 512],
                                    start=True, stop=True)
                                nc.vector.tensor_add(
                                    x_sb[:, st, nh * 512:(nh + 1) * 512],
                                    x_sb[:, st, nh * 512:(nh + 1) * 512], ps)

            # ============== Scope II: MoE (fp8 DoubleRow) ==============
            with tc.tile_pool(name="w1sp", bufs=6) as w1sp, \
                 tc.tile_pool(name="w1qp", bufs=2) as w1qp, \
                 tc.tile_pool(name="w2sp", bufs=6) as w2sp, \
                 tc.tile_pool(name="w2qp", bufs=3) as w2qp, \
                 tc.tile_pool(name="hp", bufs=1) as hp, \
                 tc.tile_pool(name="ytp", bufs=3) as ytp, \
                 tc.tile_pool(name="ps5", bufs=2, space="PSUM") as ps5:
                hT8 = hp.tile([128, FT, 512], FP8)

                # All weight-cast ops live on DVE ONLY: ACT's in-order queue
                # carries the relu/t1 stream that drains PE psums, and a
                # cast waiting on its DMA there head-of-line-blocks the PE
                # (measured ~2.6us/tile h-phase stalls). GpSimd fp8 converts
                # are ~12x slow. Deep staging pools let the DMA run several
                # blocks ahead of the casts.
                def w1_chunk(w1q, e, c):
                    """One [128 x 512] f32 chunk of W1[e] -> fp8 x W1SC.
                    kt = c//8 row-block, q = c%8 column eighth."""
                    kt, q = c // 8, c % 8
                    fs = slice(q * 512, (q + 1) * 512)
                    w1s = w1sp.tile([128, 512], F32, tag="w1s")
                    nc.sync.dma_start(
                        out=w1s, in_=W1[e, kt * 128:(kt + 1) * 128, fs])
                    nc.vector.tensor_scalar_mul(w1q[:, kt, fs], w1s, W1SC)

                def w2_block(w2h, e, nh, wc):
                    """One [128 x 512] f32 block of W2[e]'s nh d-half."""
                    w2s = w2sp.tile([128, 512], F32, tag="w2s")
                    nc.sync.dma_start(
                        out=w2s,
                        in_=W2[e, wc * 128:(wc + 1) * 128,
                               nh * 512:(nh + 1) * 512])
                    nc.vector.tensor_scalar_mul(w2h[:, wc, :], w2s, W2SC)

                def w2half():
                    w2h = w2qp.tile([128, FT, 512], FP8, tag="w2q")
                    return w2h

                # prologue: expert 0 W1 + W2 nh0-half. Staged experts load
                # fp8 straight from DRAM scratch (one big DMA, no cast).
                w1q = w1qp.tile([128, DT, F], FP8, tag="w1q")
                if NST > 0:
                    nc.sync.dma_start(out=w1q, in_=w1d[0])
                else:
                    for c in range(64):
                        w1_chunk(w1q, 0, c)
                w2a = w2half()
                if NST > 0:
                    nc.sync.dma_start(out=w2a, in_=w2d[0][0])
                else:
                    for wc in range(FT):
                        w2_block(w2a, 0, 0, wc)

                for e in range(E):
                    w2b = w2half()          # (e, nh1) half
                    if e < NST:
                        nc.sync.dma_start(out=w2b, in_=w2d[e][1])
                    w1q_next = None
                    for sh in range(2):
                        shs = slice(sh * 512, (sh + 1) * 512)
                        if sh == 1 and e < E - 1:
                            w1q_next = w1qp.tile([128, DT, F], FP8, tag="w1q")
                            if e + 1 < NST:
                                nc.sync.dma_start(out=w1q_next, in_=w1d[e + 1])
                        for ft in range(FT):
                            # prefetch interleave: one weight chunk per ft
                            if sh == 0:
                                if e >= NST:
                                    w2_block(w2b, e, 1, ft)
                            elif e < E - 1 and e + 1 >= NST:
                                w1_chunk(w1q_next, e + 1, 2 * ft)
                                w1_chunk(w1q_next, e + 1, 2 * ft + 1)
                            ps_h = ps5.tile([128, 512], F32, tag="h")
                            for kp in range(DT // 2):
                                nc.tensor.matmul(
                                    ps_h,
                                    w1q[:, 2 * kp:2 * kp + 2,
                                        ft * 128:(ft + 1) * 128],
                                    x8[:, 2 * kp:2 * kp + 2, shs],
                                    start=(kp == 0), stop=(kp == DT // 2 - 1),
                                    perf_mode=DR)
                            nc.scalar.activation(
                                hT8[:, ft, :], ps_h, AF.Relu,
                                bias=b1T[:, ft, e:e + 1], scale=1.0 / W1SC)
                        if sh == 1 and e < E - 1:
                            # (e+1, nh0) half: bunched is fine — the casts
                            # sit on DVE ahead of the y adds, which have
                            # psum slack behind ACT's t1 drain.
                            w2a_next = w2half()
                            if e + 1 < NST:
                                nc.sync.dma_start(
                                    out=w2a_next, in_=w2d[e + 1][0])
                            else:
                                for wc in range(FT):
                                    w2_block(w2a_next, e + 1, 0, wc)
                        for nh in range(2):
                            w2h = w2a if nh == 0 else w2b
                            for st2 in range(4):
                                st = sh * 4 + st2
                                ps_y = ps5.tile([128, 512], F32, tag="y")
                                for fp in range(FT // 2):
                                    nc.tensor.matmul(
                                        ps_y,
                                        hT8[:, 2 * fp:2 * fp + 2,
                                            st2 * 128:(st2 + 1) * 128],
                                        w2h[:, 2 * fp:2 * fp + 2, :],
                                        start=(fp == 0),
                                        stop=(fp == FT // 2 - 1),
                                        perf_mode=DR)
                                t1 = ytp.tile([128, 512], F32, tag="t1")
                                nc.scalar.activation(
                                    t1, ps_y, AF.Copy,
                                    scale=gate8[:, st, e:e + 1])
                                nc.vector.tensor_add(
                                    x_sb[:, st, nh * 512:(nh + 1) * 512],
                                    x_sb[:, st, nh * 512:(nh + 1) * 512], t1)
                        if e == E - 1:
                            # stream the finished half of the output out as
                            # soon as the last expert's adds for it land
                            nc.sync.dma_start(
                                out=out.ap().rearrange(
                                    "(st p) d -> p st d",
                                    p=128)[:, sh * 4:(sh + 1) * 4, :],
                                in_=x_sb[:, sh * 4:(sh + 1) * 4, :])
                    if e < E - 1:
                        w1q = w1q_next
                        w2a = w2a_next

    nc.finalize()
    return nc


_CACHE = {}


def _get_nc():
    if 'nc' not in _CACHE:
        _CACHE['nc'] = build()
    return _CACHE['nc']


def _in_maps(inputs):
    xf = np.ascontiguousarray(np.asarray(inputs['x'], dtype=np.float32))
    assert xf.shape == (B, S, D)
    nh = inputs.get('n_heads', H)
    assert int(nh) == H, f"kernel hardcodes n_heads={H}, got {nh}"
    base = {}
    for k in ('g1', 'Wq', 'Wk', 'Wv', 'Wo', 'g2', 'Wg', 'W1', 'b1', 'W2', 'b2'):
        base[k] = np.ascontiguousarray(np.asarray(inputs[k], dtype=np.float32))
    return [dict(base, x=xf[i]) for i in range(NCORES)]


def kernel(**inputs):
    nc = _get_nc()
    res = run_bass_kernel_spmd(nc, _in_maps(inputs), core_ids=list(range(NCORES)))
    return np.stack([res.results[i]['out'] for i in range(NCORES)], axis=0)


def kernel_profiled(**inputs):
    """Like kernel() but also returns neuron-profile exec_time_ns."""
    import tempfile
    nc = _get_nc()
    res = run_bass_kernel_spmd(
        nc, _in_maps(inputs), core_ids=list(range(NCORES)),
        trace=True, tmpdir=tempfile.mkdtemp())
    outv = np.stack([res.results[i]['out'] for i in range(NCORES)], axis=0)
    return outv, res.exec_time_ns


# revision 38
# speedup vs baseline: 1.0004x; 1.0004x over previous
"""Trainium2 Bass kernel for an Attention + dense-MoE transformer layer.

Distribution: pure data-parallel over the batch dim (B=8) across 8
NeuronCores — one batch element per core, weights replicated. The dense
MoE (every token through every expert, gate-weighted sum) means compute
is identical under any sharding; DP avoids all collectives.

Per-core pipeline (S=1024 tokens, D=1024, H=16 heads, F=4096, E=8):
  rmsnorm1 -> (PE-transpose) xnT -> QKV (q,k feature-major; v token-major)
  -> per-head scoresT = k_h^T.T@q_h^T -> exp (no max; values bounded)
  -> denom via ones-matmul (cross-partition sum, broadcast) -> av matmul
  -> scale by 1/denom -> Wo proj + residual -> rmsnorm2 -> x1nT
  -> gate softmax -> per-expert h=relu(x1n@W1e+b1), y=h@W2e, acc += g*(y+b2)
  -> out = acc (accumulated in-place on x1).

Attention matmuls run in bf16 (precision-sensitive: the attention output
feeds the residual stream directly). The MoE matmuls — 86% of all PE
cycles — run in fp8 e4m3 with MatmulPerfMode.DoubleRow (two k-tiles per
pass, 2x PE throughput). W1 is pre-scaled x64 and W2 x128 on cast so the
fp8 values sit in e4m3's normal range; the scales are folded back via
the relu activation scale (1/64) and the gate coefficient (gate/128).
Numpy-simulated end-to-end rel err: 1.70e-2 (gate 2e-2); the same
simulator reproduces the bf16 baseline's HW 1.257e-3 to 3 decimals.

Weights are DMA'd f32 with row-contiguous 2-4KB lines (plain
W[kt*128:(kt+1)*128, :] block slices land partition=row with no strided
rearrange), then cast on-chip: QKVO on DVE/ACT, W1/W2 fp8 casts on the
otherwise-idle GpSimd so they never head-of-line-block the relu/exp
streams. W1 is double-buffered across experts and W2 streamed as d-halves
(3 rotating buffers) so each expert's weights land before its matmuls.
Residual stream, norms, psum accum stay f32.
"""
import sys

if '/opt/trn_rl_repo' not in sys.path:
    sys.path.insert(0, '/opt/trn_rl_repo')

import numpy as np

import concourse.bass as bass
import concourse.tile as tile
from concourse import bacc, mybir
from concourse.masks import make_identity
from concourse.bass_utils import run_bass_kernel_spmd

F32 = mybir.dt.float32
BF16 = mybir.dt.bfloat16
FP8 = mybir.dt.float8e4
AX = mybir.AxisListType.X
AF = mybir.ActivationFunctionType
DR = mybir.MatmulPerfMode.DoubleRow

B, S, D, H, F, E = 8, 1024, 1024, 16, 4096, 8
DH = D // H            # 64 head dim
ST = S // 128          # 8 token tiles
DT = D // 128          # 8 feature tiles
FT = F // 128          # 32 ffn tiles
EPS = 1e-6
SCALE = DH ** -0.5     # 0.125
NCORES = 8
CH = 512               # attention s_q chunk
NCH = S // CH          # 2
W1SC = 64.0            # fp8 pre-scale for W1 (elem std 1/32 -> 2)
W2SC = 128.0           # fp8 pre-scale for W2 (elem std 1/64 -> 2)
NST = 2                # experts pre-converted to fp8 in DRAM scratch
                       # during the attention phase (its DMA is ~95% idle);
                       # each staged expert cuts MoE-phase DMA 32MB -> 8MB


def build():
    nc = bacc.Bacc("TRN2", target_bir_lowering=False)

    x = nc.declare_dram_parameter("x", [S, D], F32, isOutput=False)
    g1 = nc.declare_dram_parameter("g1", [D], F32, isOutput=False)
    Wq = nc.declare_dram_parameter("Wq", [D, D], F32, isOutput=False)
    Wk = nc.declare_dram_parameter("Wk", [D, D], F32, isOutput=False)
    Wv = nc.declare_dram_parameter("Wv", [D, D], F32, isOutput=False)
    Wo = nc.declare_dram_parameter("Wo", [D, D], F32, isOutput=False)
    g2 = nc.declare_dram_parameter("g2", [D], F32, isOutput=False)
    Wg = nc.declare_dram_parameter("Wg", [D, E], F32, isOutput=False)
    W1 = nc.declare_dram_parameter("W1", [E, D, F], F32, isOutput=False)
    b1 = nc.declare_dram_parameter("b1", [E, F], F32, isOutput=False)
    W2 = nc.declare_dram_parameter("W2", [E, F, D], F32, isOutput=False)
    b2 = nc.declare_dram_parameter("b2", [E, D], F32, isOutput=False)
    out = nc.declare_dram_parameter("out", [S, D], F32, isOutput=True)

    with tile.TileContext(nc) as tc:
        with tc.tile_pool(name="pers", bufs=1) as pers, \
             tc.tile_pool(name="x8p", bufs=1) as x8p, \
             tc.tile_pool(name="dramp", bufs=1, space="DRAM") as dramp:

            # DRAM fp8 scratch for the pre-staged experts, laid out exactly
            # as the SBUF tiles the MoE phase fills, so both the conversion
            # writes and the MoE reads use large contiguous lines.
            w1d = [dramp.tile([128, DT * F], FP8, name=f"w1d{e}")
                   for e in range(NST)]
            w2d = [[dramp.tile([128, FT * 512], FP8, name=f"w2d{e}n{nh}")
                    for nh in range(2)] for e in range(NST)]

            # ---- persistent setup ----
            x_sb = pers.tile([128, ST, D], F32)
            xr = x.ap().rearrange("(st p) d -> p st d", p=128)
            for st in range(ST):
                nc.sync.dma_start(out=x_sb[:, st, :], in_=xr[:, st, :])
            ident = pers.tile([128, 128], F32)
            make_identity(nc, ident)
            ones_bf = pers.tile([128, 128], BF16)
            nc.vector.memset(ones_bf, 1.0)
            eps_sb = pers.tile([128, 1], F32)
            nc.vector.memset(eps_sb, EPS)
            wg_sb = pers.tile([128, DT, E], BF16)
            nc.gpsimd.dma_start(out=wg_sb, in_=Wg.ap().rearrange("(kt p) e -> p kt e", p=128))
            b1T = pers.tile([128, FT, E], F32)
            gate_sb = pers.tile([128, ST, E], F32)
            gate8 = pers.tile([128, ST, E], F32)   # gate / W2SC
            x8 = x8p.tile([128, DT, S], FP8)

            # ============== Scope I: attention + gate ==============
            # Pool creation order matters: qT/kT/v die earliest (end of
            # attention core), so they sit base-most — scope II's W1/W2
            # pools land on top of them and their prefetch DMA can start
            # while the tail of scope I still runs.
            with tc.tile_pool(name="attnA", bufs=1) as attnA, \
                 tc.tile_pool(name="attnB", bufs=1) as attnB, \
                 tc.tile_pool(name="x1p", bufs=1) as x1p, \
                 tc.tile_pool(name="small", bufs=4) as small, \
                 tc.tile_pool(name="gpool", bufs=1) as gpool:
                qT = attnA.tile([128, DT, S], BF16, tag="qT")
                kT = attnA.tile([128, DT, S], BF16, tag="kT")
                v_sb = attnA.tile([128, ST, D], BF16, tag="v")
                xnT = attnB.tile([128, DT, S], BF16, tag="xT")
                x1nT = x1p.tile([128, DT, S], BF16)
                gc1 = gpool.tile([128, DT], F32)
                nc.sync.dma_start(out=gc1, in_=g1.ap().rearrange("(dt p) -> p dt", p=128))
                gc2 = gpool.tile([128, DT], F32)
                nc.sync.dma_start(out=gc2, in_=g2.ap().rearrange("(dt p) -> p dt", p=128))

                def rmsnorm_transpose(gcol, dstT, ps_tp, tmp):
                    """token-major rmsnorm of x_sb, PE-transposed into dstT
                    [128, DT, S] (bf16, feature-major); the gain g is folded
                    into the transpose drain, alternating DVE/ACT."""
                    for st in range(ST):
                        xs = x_sb[:, st, :]
                        sq = tmp.tile([128, D], F32, tag="scr")
                        ss = small.tile([128, 1], F32, tag="ss")
                        # one ACT op: squares + free-axis sum via accum_out
                        nc.scalar.activation(sq, xs, AF.Square, accum_out=ss)
                        rstd = small.tile([128, 1], F32, tag="rstd")
                        nc.scalar.activation(rstd, ss, AF.Sqrt, bias=eps_sb, scale=1.0 / D)
                        rs = small.tile([128, 1], F32, tag="rs")
                        nc.vector.reciprocal(rs, rstd)
                        xn = tmp.tile([128, D], F32, tag="scr")
                        nc.vector.tensor_scalar_mul(xn, xs, rs)
                        for dt_ in range(DT):
                            tp = ps_tp.tile([128, 128], F32, tag="tp")
                            nc.tensor.transpose(tp, xn[:, dt_ * 128:(dt_ + 1) * 128], ident)
                            dslice = dstT[:, dt_, st * 128:(st + 1) * 128]
                            if dt_ % 2 == 0:
                                nc.vector.tensor_scalar_mul(
                                    dslice, tp, gcol[:, dt_:dt_ + 1])
                            else:
                                nc.scalar.activation(
                                    dslice, tp, AF.Copy, scale=gcol[:, dt_:dt_ + 1])

                def load_wproj(wh, wpool, spool):
                    """Row-contiguous load of a [D, D] weight into bf16
                    [128, DT, D] (partition = row-in-block). 4KB DMA lines,
                    no strided rearrange."""
                    wbf = wpool.tile([128, DT, D], BF16, tag="w")
                    for kt in range(DT):
                        ws = spool.tile([128, D], F32, tag="ws")
                        nc.sync.dma_start(
                            out=ws, in_=wh[kt * 128:(kt + 1) * 128, :])
                        if kt % 2 == 0:
                            nc.vector.tensor_copy(wbf[:, kt, :], ws)
                        else:
                            nc.scalar.activation(wbf[:, kt, :], ws, AF.Copy)
                    return wbf

                # ---- rmsnorm1, b1 transpose, QKV ----
                with tc.tile_pool(name="ps12", bufs=3, space="PSUM") as ps12, \
                     tc.tile_pool(name="wpp", bufs=2) as wpp, \
                     tc.tile_pool(name="wstg", bufs=4) as wstg, \
                     tc.tile_pool(name="tmp1", bufs=3) as tmp1, \
                     tc.tile_pool(name="b1rp", bufs=1) as b1rp:
                    # b1 -> b1T via PE transpose (8 contiguous DMA lines
                    # instead of 32K 4-byte strided lines)
                    b1row = b1rp.tile([E, F], F32)
                    nc.gpsimd.dma_start(out=b1row, in_=b1.ap())
                    for ft in range(FT):
                        tpb = ps12.tile([128, 128], F32, tag="tp")
                        nc.tensor.transpose(
                            tpb[:, 0:E], b1row[:, ft * 128:(ft + 1) * 128],
                            ident[0:E, 0:E])
                        nc.vector.tensor_copy(b1T[:, ft, :], tpb[:, 0:E])

                    rmsnorm_transpose(gc1, xnT, ps12, tmp1)

                    for wh, dstT in ((Wq, qT), (Wk, kT)):
                        wbf = load_wproj(wh, wpp, wstg)
                        for mt in range(DT):
                            for nh in range(2):
                                ps = ps12.tile([128, 512], F32, tag="mm")
                                for kt in range(DT):
                                    nc.tensor.matmul(
                                        ps, wbf[:, kt, mt * 128:(mt + 1) * 128],
                                        xnT[:, kt, nh * 512:(nh + 1) * 512],
                                        start=(kt == 0), stop=(kt == DT - 1))
                                dsl = dstT[:, mt, nh * 512:(nh + 1) * 512]
                                if (2 * mt + nh) % 2 == 0:
                                    nc.vector.tensor_copy(dsl, ps)
                                else:
                                    nc.scalar.activation(dsl, ps, AF.Copy)

                    wv_c = load_wproj(Wv, wpp, wstg)
                    for nh in range(2):
                        for st in range(ST):
                            ps = ps12.tile([128, 512], F32, tag="mm")
                            for kt in range(DT):
                                nc.tensor.matmul(
                                    ps, xnT[:, kt, st * 128:(st + 1) * 128],
                                    wv_c[:, kt, nh * 512:(nh + 1) * 512],
                                    start=(kt == 0), stop=(kt == DT - 1))
                            vsl = v_sb[:, st, nh * 512:(nh + 1) * 512]
                            if st % 2 == 0:
                                nc.vector.tensor_copy(vsl, ps)
                            else:
                                nc.scalar.activation(vsl, ps, AF.Copy)

                # ---- attention core ----
                # Head PAIRS (2t, 2t+1) share one 128-row tile of qT/kT:
                # even head in partitions 0-63, odd in 64-127. dn/av stack
                # the pair on psum row groups 0/64. Software-pipelined:
                # iteration i's scores are emitted before iteration i-1's
                # dn/av so the PE never waits on ACT's exp.
                avT = attnB.tile([128, DT, S], BF16, tag="xT")  # reuses xnT
                with tc.tile_pool(name="wop", bufs=1) as wop, \
                     tc.tile_pool(name="wstg2", bufs=2) as wstg2:
                    with tc.tile_pool(name="ps3", bufs=2, space="PSUM") as ps3, \
                         tc.tile_pool(name="expp", bufs=2) as expp, \
                         tc.tile_pool(name="recp", bufs=2) as recp, \
                         tc.tile_pool(name="cstg", bufs=3) as cstg, \
                         tc.tile_pool(name="cq8", bufs=3) as cq8:
                        # Wo load emitted first: its DMA+casts overlap the
                        # attention core, so the Wo matmuls start immediately
                        # after the last av drain.
                        wo_c = load_wproj(Wo, wop, wstg2)

                        # fp8 pre-staging of experts 0..NST-1 into DRAM
                        # scratch: f32 in, DVE cast, fp8 out. Emitted a few
                        # units per attention iteration so the casts never
                        # delay the PE-critical recip/mul drain by much.
                        # Conversion DMA rides the GpSimd trigger queue:
                        # the sync queue carries the MoE weight stream, and
                        # an in-order queue would make the MoE prologue wait
                        # behind any conversion traffic that outlives the
                        # attention core.
                        def conv_w1_unit(e, c):
                            kt, q = c // 4, c % 4
                            fs = slice(q * 1024, (q + 1) * 1024)
                            cs = cstg.tile([128, 1024], F32, tag="cs", name="cs")
                            nc.gpsimd.dma_start(
                                out=cs, in_=W1[e, kt * 128:(kt + 1) * 128, fs])
                            c8 = cq8.tile([128, 1024], FP8, tag="c8", name="c8")
                            nc.vector.tensor_scalar_mul(c8, cs, W1SC)
                            nc.gpsimd.dma_start(
                                out=w1d[e][:, kt * F + q * 1024:
                                           kt * F + (q + 1) * 1024], in_=c8)

                        def conv_w2_unit(e, wc):
                            cs = cstg.tile([128, 1024], F32, tag="cs", name="cs")
                            nc.gpsimd.dma_start(
                                out=cs, in_=W2[e, wc * 128:(wc + 1) * 128, :])
                            c8 = cq8.tile([128, 1024], FP8, tag="c8", name="c8")
                            nc.vector.tensor_scalar_mul(c8, cs, W2SC)
                            for nh2 in range(2):
                                nc.gpsimd.dma_start(
                                    out=w2d[e][nh2][:, wc * 512:(wc + 1) * 512],
                                    in_=c8[:, nh2 * 512:(nh2 + 1) * 512])

                        conv_units = []
                        for e in range(NST):
                            conv_units += [(conv_w1_unit, e, c) for c in range(32)]
                            conv_units += [(conv_w2_unit, e, wc) for wc in range(FT)]
                        conv_pos = 0

                        def conv_step(n):
                            nonlocal conv_pos
                            for fn, e, i in conv_units[conv_pos:conv_pos + n]:
                                fn(e, i)
                            conv_pos += n

                        def attn_drain(state):
                            t, cs, exp_e, exp_o = state
                            ps_dn = ps3.tile([128, CH], F32, tag="dn")
                            for kt in range(ST):
                                nc.tensor.matmul(
                                    ps_dn[0:64, :], ones_bf[:, 0:64], exp_e[:, kt, :],
                                    start=(kt == 0), stop=(kt == ST - 1))
                                nc.tensor.matmul(
                                    ps_dn[64:128, :], ones_bf[:, 64:128], exp_o[:, kt, :],
                                    start=(kt == 0), stop=(kt == ST - 1))
                            recipb = recp.tile([128, CH], F32, tag="recip")
                            nc.vector.reciprocal_approx_fast(recipb, ps_dn)
                            ps_av = ps3.tile([128, CH], F32, tag="av")
                            for kt in range(ST):
                                nc.tensor.matmul(
                                    ps_av[0:64, :],
                                    v_sb[:, kt, (2 * t) * 64:(2 * t) * 64 + 64],
                                    exp_e[:, kt, :],
                                    start=(kt == 0), stop=(kt == ST - 1))
                                nc.tensor.matmul(
                                    ps_av[64:128, :],
                                    v_sb[:, kt, (2 * t + 1) * 64:(2 * t + 1) * 64 + 64],
                                    exp_o[:, kt, :],
                                    start=(kt == 0), stop=(kt == ST - 1))
                            nc.vector.tensor_mul(avT[:, t, cs], ps_av, recipb)

                        prev = None
                        for t in range(H // 2):
                            for c in range(NCH):
                                cs = slice(c * CH, (c + 1) * CH)
                                exp_e = expp.tile([128, ST, CH], BF16, tag="expe")
                                exp_o = expp.tile([128, ST, CH], BF16, tag="expo")
                                for kt in range(ST):
                                    ks = slice(kt * 128, (kt + 1) * 128)
                                    ps_e = ps3.tile([128, CH], F32, tag="sce")
                                    nc.tensor.matmul(
                                        ps_e, kT[0:64, t, ks], qT[0:64, t, cs],
                                        start=True, stop=True)
                                    ps_o = ps3.tile([128, CH], F32, tag="sco")
                                    nc.tensor.matmul(
                                        ps_o, kT[64:128, t, ks], qT[64:128, t, cs],
                                        start=True, stop=True)
                                    nc.scalar.activation(
                                        exp_e[:, kt, :], ps_e, AF.Exp, scale=SCALE)
                                    nc.scalar.activation(
                                        exp_o[:, kt, :], ps_o, AF.Exp, scale=SCALE)
                                cur = (t, cs, exp_e, exp_o)
                                if prev is not None:
                                    attn_drain(prev)
                                conv_step((len(conv_units) + 31) // 32)
                                prev = cur
                        attn_drain(prev)
                        conv_step(len(conv_units) - conv_pos)

                    # ---- Wo proj + residual, rmsnorm2, gate ----
                    with tc.tile_pool(name="ps4", bufs=3, space="PSUM") as ps4, \
                         tc.tile_pool(name="tmp2", bufs=3) as tmp2:
                        for nh in range(2):
                            for st in range(ST):
                                ps = ps4.tile([128, 512], F32, tag="mm")
                                for kt in range(DT):
                                    nc.tensor.matmul(
                                        ps, avT[:, kt, st * 128:(st + 1) * 128],
                                        wo_c[:, kt, nh * 512:(nh + 1) * 512],
                                        start=(kt == 0), stop=(kt == DT - 1))
                                nc.vector.tensor_add(
                                    x_sb[:, st, nh * 512:(nh + 1) * 512],
                                    x_sb[:, st, nh * 512:(nh + 1) * 512], ps)

                        rmsnorm_transpose(gc2, x1nT, ps4, tmp2)

                        # gate = softmax(x1n @ Wg) token-major [128, st, E]
                        for st in range(ST):
                            ps = ps4.tile([128, 512], F32, tag="mm")
                            for kt in range(DT):
                                nc.tensor.matmul(
                                    ps[:, :E], x1nT[:, kt, st * 128:(st + 1) * 128],
                                    wg_sb[:, kt, :],
                                    start=(kt == 0), stop=(kt == DT - 1))
                            gexp = small.tile([128, E], F32, tag="gexp")
                            nc.scalar.activation(gexp, ps[:, :E], AF.Exp)
                            gsum = small.tile([128, 1], F32, tag="gsum")
                            nc.vector.reduce_sum(gsum, gexp, axis=AX)
                            grec = small.tile([128, 1], F32, tag="grec")
                            nc.vector.reciprocal(grec, gsum)
                            nc.vector.tensor_scalar_mul(gate_sb[:, st, :], gexp, grec)
                            nc.vector.tensor_scalar_mul(
                                gate8[:, st, :], gate_sb[:, st, :], 1.0 / W2SC)

                        # x1nT's last reader is the gate matmul; cast to fp8
                        # now so the x1p region frees early for scope II.
                        nc.vector.tensor_copy(x8, x1nT)

                        # out += gate @ b2 (handles Sum_e g_e*b2_e once)
                        b2rb = gpool.tile([8, D], BF16)
                        nc.gpsimd.dma_start(out=b2rb, in_=b2.ap())
                        gateT = gpool.tile([8, ST, 128], BF16)
                        for st in range(ST):
                            tpg = ps4.tile([128, 128], F32, tag="tp")
                            nc.tensor.transpose(
                                tpg[:8, :], gate_sb[:, st, :], ident)
                            nc.vector.tensor_copy(gateT[:, st, :], tpg[:8, :])
                        for st in range(ST):
                            for nh in range(2):
                                ps = ps4.tile([128, 512], F32, tag="mm")
                                nc.tensor.matmul(
                                    ps, gateT[:, st, :],
                                    b2rb[:, nh * 512:(nh + 1) * 512],
                                    start=True, stop=True)
                                nc.vector.tensor_add(
                                    x_sb[:, st, nh * 512:(nh + 1) * 512],
                                    x_sb[:, st, nh * 512:(nh + 1) * 512], ps)

            # ============== Scope II: MoE (fp8 DoubleRow) ==============
            with tc.tile_pool(name="w1sp", bufs=6) as w1sp, \
                 tc.tile_pool(name="w1qp", bufs=2) as w1qp, \
                 tc.tile_pool(name="w2sp", bufs=6) as w2sp, \
                 tc.tile_pool(name="w2qp", bufs=3) as w2qp, \
                 tc.tile_pool(name="hp", bufs=1) as hp, \
                 tc.tile_pool(name="ytp", bufs=3) as ytp, \
                 tc.tile_pool(name="ps5", bufs=2, space="PSUM") as ps5:
                hT8 = hp.tile([128, FT, 512], FP8)

                # All weight-cast ops live on DVE ONLY: ACT's in-order queue
                # carries the relu/t1 stream that drains PE psums, and a
                # cast waiting on its DMA there head-of-line-blocks the PE
                # (measured ~2.6us/tile h-phase stalls). GpSimd fp8 converts
                # are ~12x slow. Deep staging pools let the DMA run several
                # blocks ahead of the casts.
                def w1_chunk(w1q, e, c):
                    """One [128 x 512] f32 chunk of W1[e] -> fp8 x W1SC.
                    kt = c//8 row-block, q = c%8 column eighth."""
                    kt, q = c // 8, c % 8
                    fs = slice(q * 512, (q + 1) * 512)
                    w1s = w1sp.tile([128, 512], F32, tag="w1s")
                    nc.sync.dma_start(
                        out=w1s, in_=W1[e, kt * 128:(kt + 1) * 128, fs])
                    nc.vector.tensor_scalar_mul(w1q[:, kt, fs], w1s, W1SC)

                def w2_block(w2h, e, nh, wc):
                    """One [128 x 512] f32 block of W2[e]'s nh d-half."""
                    w2s = w2sp.tile([128, 512], F32, tag="w2s")
                    nc.sync.dma_start(
                        out=w2s,
                        in_=W2[e, wc * 128:(wc + 1) * 128,
                               nh * 512:(nh + 1) * 512])
                    nc.vector.tensor_scalar_mul(w2h[:, wc, :], w2s, W2SC)

                def w2half():
                    w2h = w2qp.tile([128, FT, 512], FP8, tag="w2q")
                    return w2h

                # prologue: expert 0 W1 + W2 nh0-half. Staged experts load
                # fp8 straight from DRAM scratch (one big DMA, no cast).
                w1q = w1qp.tile([128, DT, F], FP8, tag="w1q")
                if NST > 0:
                    nc.sync.dma_start(out=w1q, in_=w1d[0])
                else:
                    for c in range(64):
                        w1_chunk(w1q, 0, c)
                w2a = w2half()
                if NST > 0:
                    nc.sync.dma_start(out=w2a, in_=w2d[0][0])
                else:
                    for wc in range(FT):
                        w2_block(w2a, 0, 0, wc)

                for e in range(E):
                    w2b = w2half()          # (e, nh1) half
                    if e < NST:
                        nc.sync.dma_start(out=w2b, in_=w2d[e][1])
                    w1q_next = None
                    for sh in range(2):
                        shs = slice(sh * 512, (sh + 1) * 512)
                        if sh == 1 and e < E - 1:
                            w1q_next = w1qp.tile([128, DT, F], FP8, tag="w1q")
                            if e + 1 < NST:
                                nc.sync.dma_start(out=w1q_next, in_=w1d[e + 1])
                        for ft in range(FT):
                            # prefetch interleave: one weight chunk per ft
                            if sh == 0:
                                if e >= NST:
                                    w2_block(w2b, e, 1, ft)
                            elif e < E - 1 and e + 1 >= NST:
                                w1_chunk(w1q_next, e + 1, 2 * ft)
                                w1_chunk(w1q_next, e + 1, 2 * ft + 1)
                            ps_h = ps5.tile([128, 512], F32, tag="h", bufs=3)
                            for kp in range(DT // 2):
                                nc.tensor.matmul(
                                    ps_h,
                                    w1q[:, 2 * kp:2 * kp + 2,
                                        ft * 128:(ft + 1) * 128],
                                    x8[:, 2 * kp:2 * kp + 2, shs],
                                    start=(kp == 0), stop=(kp == DT // 2 - 1),
                                    perf_mode=DR)
                            nc.scalar.activation(
                                hT8[:, ft, :], ps_h, AF.Relu,
                                bias=b1T[:, ft, e:e + 1], scale=1.0 / W1SC)
                        if sh == 1 and e < E - 1:
                            # (e+1, nh0) half: bunched is fine — the casts
                            # sit on DVE ahead of the y adds, which have
                            # psum slack behind ACT's t1 drain.
                            w2a_next = w2half()
                            if e + 1 < NST:
                                nc.sync.dma_start(
                                    out=w2a_next, in_=w2d[e + 1][0])
                            else:
                                for wc in range(FT):
                                    w2_block(w2a_next, e + 1, 0, wc)
                        for nh in range(2):
                            w2h = w2a if nh == 0 else w2b
                            for st2 in range(4):
                                st = sh * 4 + st2
                                ps_y = ps5.tile([128, 512], F32, tag="y")
                                for fp in range(FT // 2):
                                    nc.tensor.matmul(
                                        ps_y,
                                        hT8[:, 2 * fp:2 * fp + 2,
                                            st2 * 128:(st2 + 1) * 128],
                                        w2h[:, 2 * fp:2 * fp + 2, :],
                                        start=(fp == 0),
                                        stop=(fp == FT // 2 - 1),
                                        perf_mode=DR)
                                t1 = ytp.tile([128, 512], F32, tag="t1")
                                nc.scalar.activation(
                                    t1, ps_y, AF.Copy,
                                    scale=gate8[:, st, e:e + 1])
                                nc.vector.tensor_add(
                                    x_sb[:, st, nh * 512:(nh + 1) * 512],
                                    x_sb[:, st, nh * 512:(nh + 1) * 512], t1)
                        if e == E - 1:
                            # stream the finished half of the output out as
                            # soon as the last expert's adds for it land
                            nc.sync.dma_start(
                                out=out.ap().rearrange(
                                    "(st p) d -> p st d",
                                    p=128)[:, sh * 4:(sh + 1) * 4, :],
                                in_=x_sb[:, sh * 4:(sh + 1) * 4, :])
                    if e < E - 1:
                        w1q = w1q_next
                        w2a = w2a_next

    nc.finalize()
    return nc


_CACHE = {}


def _get_nc():
    if 'nc' not in _CACHE:
        _CACHE['nc'] = build()
    return _CACHE['nc']


def _in_maps(inputs):
    xf = np.ascontiguousarray(np.asarray(inputs['x'], dtype=np.float32))
    assert xf.shape == (B, S, D)
    nh = inputs.get('n_heads', H)
    assert int(nh) == H, f"kernel hardcodes n_heads={H}, got {nh}"
    base = {}
    for k in ('g1', 'Wq', 'Wk', 'Wv', 'Wo', 'g2', 'Wg', 'W1', 'b1', 'W2', 'b2'):
        base[k] = np.ascontiguousarray(np.asarray(inputs[k], dtype=np.float32))
    return [dict(base, x=xf[i]) for i in range(NCORES)]


def kernel(**inputs):
    nc = _get_nc()
    res = run_bass_kernel_spmd(nc, _in_maps(inputs), core_ids=list(range(NCORES)))
    return np.stack([res.results[i]['out'] for i in range(NCORES)], axis=0)


def kernel_profiled(**inputs):
    """Like kernel() but also returns neuron-profile exec_time_ns."""
    import tempfile
    nc = _get_nc()
    res = run_bass_kernel_spmd(
        nc, _in_maps(inputs), core_ids=list(range(NCORES)),
        trace=True, tmpdir=tempfile.mkdtemp())
    outv = np.stack([res.results[i]['out'] for i in range(NCORES)], axis=0)
    return outv, res.exec_time_ns
